# revision 1
# baseline (speedup 1.0000x reference)
"""Trainium2 Bass kernel for nn_Cross_IAN.

The reference computes
    eij = 0.5*softmax(s11, -1) + 0.5*softmax(s12, -1)   # [B,S,S]
    eij = mean(eij, axis=2, keepdims=True)              # [B,S,1]
    out = sum(x0 * eij, axis=1)                         # [B,D]
The mean is taken over the same axis the softmaxes normalize, so every
row of each softmax sums to exactly 1 and eij == 1/S identically --
independent of x1, W1, W2.  The output is exactly mean(x0, axis=1),
a pure reduction over the sequence axis of x0.

Kernel strategy (pure data parallel over batch, 8 batches/core):
  - per batch: two DMAs load [512, 768] row-blocks as [128, 4, 768] SBUF
    tiles (each partition line = 4 contiguous rows = 12KB contiguous DRAM)
  - in-place DVE pairwise adds reduce the q axis -> [128, 768] per batch
  - one fp32 matmul per PSUM half with a one-hot (1/1024)-scaled [128,8]
    column reduces the partition axis into PSUM row b; all batches
    accumulate into one [8, 384] pair of PSUM tiles
  - PSUM copied to SBUF once, single DMA out

The walrus build in this container lowers at most ONE sync wait per
instruction, so the dependency graph is shaped so every instruction
carries a single cross-engine wait:
  - input DMAs go on SWDGE lanes round-robin (8 lanes); with 2 DMAs per
    batch and 4 slots per input pool, a recycled slot's previous DMA sits
    exactly 8 DMAs earlier = the SAME lane, so its WAW doubles as the
    built-in same-lane throttle wait (the one allowed wait per DMA)
  - a 1-element Pool-engine relay read of the accumulator from bufs
    batches ago carries the WAR wait for the recycled input slots; its
    wait executes on the Pool sequencer, so the following dma_starts
    need no WAR wait of their own
  - each DVE add reads tiles from at most one DMA; cross-tile adds read
    only DVE-written slices (same-engine program order, no waits)
  - the accumulator pool has one slot per batch (no reuse -> no WAR)
  - Tile's kernel-tail drain waits on the whole global clock; it is
    post-processed into a chain of single-wait drains
"""

from contextlib import ExitStack

import numpy as np

import concourse.bass as bass
import concourse.tile as tile
from concourse import mybir
from concourse.bass_utils import run_bass_kernel_spmd

B, S, D = 64, 1024, 768
N_CORES = 8
B_PER = B // N_CORES  # 8 batches per core
P = 128               # SBUF partitions
Q = S // P            # 8 sequence rows folded into each partition line
DPB = 2               # DMAs per batch
QS = Q // DPB         # q-rows per DMA tile
HALF = D // 2         # 384, fits one PSUM bank in fp32
IN_BUFS = 4           # slots per input pool; reuse distance 8 DMAs = 8 lanes

_CACHE = {}


def _build() -> bass.Bass:
    nc = bass.Bass(trn_type="TRN2")
    x = nc.declare_dram_parameter("x", [B_PER, S, D], mybir.dt.float32, isOutput=False)
    y = nc.declare_dram_parameter("y", [B_PER, D], mybir.dt.float32, isOutput=True)

    with tile.TileContext(nc) as tc, ExitStack() as ctx:
        pools = [
            ctx.enter_context(tc.tile_pool(name=f"in{k}", bufs=IN_BUFS))
            for k in range(DPB)
        ]
        acc_pool = ctx.enter_context(tc.tile_pool(name="acc", bufs=B_PER))
        psum_pool = ctx.enter_context(tc.tile_pool(name="psum", bufs=1, space="PSUM"))
        const_pool = ctx.enter_context(tc.tile_pool(name="const", bufs=1))
        out_pool = ctx.enter_context(tc.tile_pool(name="out", bufs=1))

        # One-hot reduction matrices: eye[:, b, m] = (1/S) * (m == b).
        eye = const_pool.tile([P, B_PER, B_PER], mybir.dt.float32)
        nc.vector.memset(eye[:], 0.0)
        for b in range(B_PER):
            nc.vector.memset(eye[:, b, b : b + 1], 1.0 / S)

        ps0 = psum_pool.tile([B_PER, HALF], mybir.dt.float32)
        ps1 = psum_pool.tile([B_PER, HALF], mybir.dt.float32)
        scr0 = const_pool.tile([1, B_PER], mybir.dt.float32)

        accs = []
        for b in range(B_PER):
            xb = x[b].rearrange("(p q) d -> p q d", p=P)
            if b >= IN_BUFS:
                # Pool-engine relay (see module docstring)
                nc.gpsimd.tensor_copy(
                    out=scr0[0:1, b : b + 1], in_=accs[b - IN_BUFS][0:1, 0:1]
                )
            ts = []
            for k in range(DPB):
                t = pools[k].tile([P, QS, D], mybir.dt.float32, tag=f"in{k}")
                nc.gpsimd.dma_start(out=t[:], in_=xb[:, k * QS : (k + 1) * QS, :])
                ts.append(t)
            # within-tile reduction (in-place; deps on a single DMA each)
            for t in ts:
                w = QS
                while w > 1:
                    nc.vector.tensor_add(
                        t[:, 0 : w // 2, :], t[:, 0 : w // 2, :], t[:, w // 2 : w, :]
                    )
                    w //= 2
            # cross-tile tree over DVE-written slices only
            while len(ts) > 2:
                nxt = []
                for k in range(0, len(ts), 2):
                    nc.vector.tensor_add(
                        ts[k][:, 0, :], ts[k][:, 0, :], ts[k + 1][:, 0, :]
                    )
                    nxt.append(ts[k])
                ts = nxt
            a = acc_pool.tile([P, D], mybir.dt.float32, tag="a")
            nc.vector.tensor_add(a[:], ts[0][:, 0, :], ts[1][:, 0, :])
            accs.append(a)

            start, stop = b == 0, b == B_PER - 1
            nc.tensor.matmul(
                ps0[:], lhsT=eye[:, b, :], rhs=a[:, 0:HALF], start=start, stop=stop
            )
            nc.tensor.matmul(
                ps1[:], lhsT=eye[:, b, :], rhs=a[:, HALF:D], start=start, stop=stop
            )

        out_t = out_pool.tile([B_PER, D], mybir.dt.float32)
        nc.vector.tensor_copy(out=out_t[:, 0:HALF], in_=ps0[:])
        nc.vector.tensor_copy(out=out_t[:, HALF:D], in_=ps1[:])
        nc.sync.dma_start(out=y[:], in_=out_t[:])

    _split_multiwait_drains(nc)
    return nc


def _split_multiwait_drains(nc: bass.Bass) -> None:
    """walrus lowers at most one sync wait per instruction; Tile's kernel-tail
    drain waits on the whole global clock.  Split it into a chain of
    single-wait drains (a drain with nothing new pending is a no-op, and the
    SP sequencer executes the waits in order, which is equivalent)."""
    for blk in nc.m.functions[0].blocks:
        insts = blk.instructions
        k = 0
        while k < len(insts):
            i = insts[k]
            si = i.sync_info
            if si is not None and len(si.on_wait) > 1:
                assert type(i).__name__ == "InstDrain", (i.name, type(i).__name__)
                waits = list(si.on_wait)
                for j, w in enumerate(waits[:-1]):
                    nd = mybir.InstDrain(
                        name=f"{i.name}-wsplit{j}", engine=i.engine, ins=[], outs=[]
                    )
                    nd.sync_info = mybir.SyncInfo(on_wait=[w], on_update=[])
                    nc.register_instruction(nd, overwrite=True)
                    insts.insert(k + j, nd)
                i.sync_info = mybir.SyncInfo(
                    on_wait=[waits[-1]], on_update=list(si.on_update)
                )
                k += len(waits) - 1
            k += 1


def _shards(x0: np.ndarray) -> list[dict[str, np.ndarray]]:
    return [
        {"x": np.ascontiguousarray(x0[i * B_PER : (i + 1) * B_PER])}
        for i in range(N_CORES)
    ]


def kernel(**inputs: np.ndarray) -> np.ndarray:
    x0 = np.asarray(inputs["x0"], dtype=np.float32)
    if "nc" not in _CACHE:
        _CACHE["nc"] = _build()
    res = run_bass_kernel_spmd(_CACHE["nc"], _shards(x0), core_ids=list(range(N_CORES)))
    return np.concatenate([r["y"] for r in res.results], axis=0)



# revision 16
# speedup vs baseline: 1.0924x; 1.0924x over previous
"""Trainium2 Bass kernel for nn_Cross_IAN.

The reference computes
    eij = 0.5*softmax(s11, -1) + 0.5*softmax(s12, -1)   # [B,S,S]
    eij = mean(eij, axis=2, keepdims=True)              # [B,S,1]
    out = sum(x0 * eij, axis=1)                         # [B,D]
The mean is taken over the same axis the softmaxes normalize, so every
row of each softmax sums to exactly 1 and eij == 1/S identically --
independent of x1, W1, W2.  The output is exactly mean(x0, axis=1),
a pure reduction over the sequence axis of x0.

Kernel strategy (pure data parallel over batch, 8 batches/core).  The
DMA bus (360 GB/s/core) is the roofline: 25.2MB of x0 per core = 69.9us
of transfer.  Everything else is scheduled to hide under it:

  - per batch, rows are folded 8-to-a-partition ([128, 8, 768]); the
    first 7 row-planes load early as [128,4,768] + [128,3,768] tiles,
    are tree-reduced on DVE into a [128,768] acc, and accumulated into
    PSUM rows via one-hot (1/1024) matmuls (fp32r: 1 cycle/row)
  - the LAST row-plane of every batch is deferred: 9 small "sliver"
    DMAs ([128,1,768] for batches 0-6, two [128,1,384] halves for
    batch 7) stream at the very end, each consumed DIRECTLY by fp32r
    matmuls accumulating into PSUM -- no DVE pass on the critical path
  - ps0 (cols 0:384) closes one sliver earlier than ps1 (cols 384:768);
    each half is DMAd to DRAM straight out of PSUM on its own HWDGE
    engine (Activation / SP), so the kernel tail after the last input
    transfer is just: 900ns DMA sem prop + one 384-wide fp32r matmul +
    HWDGE gen + DGE delay + 34ns transfer + 900ns sem prop + drains.

The walrus build in this container lowers at most ONE sync wait per
instruction, so the dependency graph is shaped so every instruction
carries a single cross-engine wait:
  - input DMAs go on SWDGE lanes round-robin; with 2 DMAs per batch and
    4 slots per main-input pool, a recycled slot's previous DMA sits
    exactly 8 DMAs earlier = the SAME lane, so its WAW doubles as the
    built-in same-lane throttle wait (the one allowed wait per DMA)
  - a 1-element Pool-engine relay read of the accumulator from bufs
    batches ago carries the WAR wait for the recycled input slots
  - sliver tiles are single-use (9 fresh slots -> no WAW/WAR waits)
  - each sliver matmul's one wait is its sliver's DMA sem; partial
    matmuls' one wait is the DVE sem of their acc
  - Tile's kernel-tail drain waits on the whole global clock; it is
    post-processed into a chain of single-wait drains
"""

from contextlib import ExitStack

import numpy as np

import concourse.bass as bass
import concourse.tile as tile
from concourse import mybir
from concourse.bass_utils import run_bass_kernel_spmd

B, S, D = 64, 1024, 768
N_CORES = 8
B_PER = B // N_CORES  # 8 batches per core
P = 128               # SBUF partitions
Q = S // P            # 8 sequence rows folded into each partition line
HALF = D // 2         # 384, fits one PSUM bank in fp32
Q1, Q2 = 4, 3         # main-load row-planes per batch (plane 7 is the sliver)
IN_BUFS = 4           # slots per main input pool; reuse distance 8 DMAs = 8 lanes
F32R = mybir.dt.float32r

_CACHE = {}


def _build() -> bass.Bass:
    nc = bass.Bass(trn_type="TRN2")
    x = nc.declare_dram_parameter("x", [B_PER, S, D], mybir.dt.float32, isOutput=False)
    y = nc.declare_dram_parameter("y", [B_PER, D], mybir.dt.float32, isOutput=True)

    with tile.TileContext(nc) as tc, ExitStack() as ctx:
        pools = [
            ctx.enter_context(tc.tile_pool(name=f"in{k}", bufs=IN_BUFS))
            for k in range(2)
        ]
        sl_pool = ctx.enter_context(tc.tile_pool(name="sl", bufs=1))
        acc_pool = ctx.enter_context(tc.tile_pool(name="acc", bufs=B_PER))
        psum_pool = ctx.enter_context(tc.tile_pool(name="psum", bufs=1, space="PSUM"))
        const_pool = ctx.enter_context(tc.tile_pool(name="const", bufs=1))

        # One-hot reduction matrices: eye[:, b, m] = (1/S) * (m == b).
        # The fp32 memset image is round-converted into an fp32r copy on DVE
        # (the walrus BIR verifier requires fp32r matmul operands to come
        # from an instruction that rounds to fp32r).
        eye_f = const_pool.tile([P, B_PER, B_PER], mybir.dt.float32)
        nc.vector.memset(eye_f[:], 0.0)
        for b in range(B_PER):
            nc.vector.memset(eye_f[:, b, b : b + 1], 1.0 / S)
        eye = const_pool.tile([P, B_PER, B_PER], F32R)
        nc.vector.tensor_copy(out=eye[:], in_=eye_f[:])

        ps0 = psum_pool.tile([B_PER, HALF], mybir.dt.float32)
        ps1 = psum_pool.tile([B_PER, HALF], mybir.dt.float32)
        scr0 = const_pool.tile([1, B_PER], F32R)

        def mm(ps, b, rhs, start, stop):
            nc.tensor.matmul(
                ps[:],
                lhsT=eye[:, b, :],
                rhs=rhs,
                start=start,
                stop=stop,
            )

        # ---- phase 1: main loads (row-planes 0:7) + DVE trees + partial mms
        accs = []
        for b in range(B_PER):
            xb = x[b].rearrange("(p q) d -> p q d", p=P)
            if b >= IN_BUFS:
                # Pool-engine relay (see module docstring)
                nc.gpsimd.tensor_copy(
                    out=scr0[0:1, b : b + 1], in_=accs[b - IN_BUFS][0:1, 0:1]
                )
            t1 = pools[0].tile([P, Q1, D], mybir.dt.float32, tag="in0")
            nc.gpsimd.dma_start(out=t1[:], in_=xb[:, 0:Q1, :])
            t2 = pools[1].tile([P, Q2, D], mybir.dt.float32, tag="in1")
            nc.gpsimd.dma_start(out=t2[:], in_=xb[:, Q1 : Q1 + Q2, :])
            # in-place tree reductions (each add depends on a single DMA)
            nc.vector.tensor_add(t1[:, 0:2, :], t1[:, 0:2, :], t1[:, 2:4, :])
            nc.vector.tensor_add(t1[:, 0, :], t1[:, 0, :], t1[:, 1, :])
            nc.vector.tensor_add(t2[:, 0, :], t2[:, 0, :], t2[:, 2, :])
            nc.vector.tensor_add(t2[:, 0, :], t2[:, 0, :], t2[:, 1, :])
            a = acc_pool.tile([P, D], F32R, tag="a")
            nc.vector.tensor_add(a[:], t1[:, 0, :], t2[:, 0, :])
            accs.append(a)
            if b < B_PER - 1:
                # batch 7's partial mms are deferred into the sliver phase:
                # its acc lands at ~67.6us and would gate the early sliver mms
                mm(ps0, b, a[:, 0:HALF], start=b == 0, stop=False)
                mm(ps1, b, a[:, HALF:D], start=b == 0, stop=False)

        # ---- phase 2: sliver loads (row-plane 7), batch 7 split in halves
        slivers = []
        for b in range(B_PER - 1):
            xb = x[b].rearrange("(p q) d -> p q d", p=P)
            t = sl_pool.tile([P, 1, D], F32R, tag=f"sl{b}")
            nc.gpsimd.dma_start(out=t[:], in_=xb[:, Q - 1 : Q, :].bitcast(F32R))
            slivers.append(t)
        x7 = x[B_PER - 1].rearrange("(p q) d -> p q d", p=P)
        s70 = sl_pool.tile([P, 1, HALF], F32R, tag="sl70")
        nc.gpsimd.dma_start(out=s70[:], in_=x7[:, Q - 1 : Q, 0:HALF].bitcast(F32R))
        s71 = sl_pool.tile([P, 1, HALF], F32R, tag="sl71")
        nc.gpsimd.dma_start(out=s71[:], in_=x7[:, Q - 1 : Q, HALF:D].bitcast(F32R))

        # ---- phase 3: sliver matmuls straight off the DMA'd tiles, ordered
        # purely by data arrival (slivers b0..b5, then batch 7's deferred
        # partials whose acc lands mid-stream, then the late slivers) so the
        # PE matmul stream starts at the first sliver and the pstate model
        # promotes the tail matmuls to the fast rates.
        for b in range(6):
            mm(ps0, b, slivers[b][:, 0, 0:HALF], start=False, stop=False)
            mm(ps1, b, slivers[b][:, 0, HALF:D], start=False, stop=False)
        a7 = accs[B_PER - 1]
        mm(ps0, B_PER - 1, a7[:, 0:HALF], start=False, stop=False)
        mm(ps1, B_PER - 1, a7[:, HALF:D], start=False, stop=False)
        mm(ps0, 6, slivers[6][:, 0, 0:HALF], start=False, stop=False)
        mm(ps1, 6, slivers[6][:, 0, HALF:D], start=False, stop=False)
        mm(ps0, B_PER - 1, s70[:, 0, :], start=False, stop=True)
        mm(ps1, B_PER - 1, s71[:, 0, :], start=False, stop=True)

        # ---- phase 4: both PSUM halves copied on DVE (ps0's copy overlaps
        # the final matmul; the single out-DMA then carries ONE wait, on the
        # DVE sem of the second copy, which transitively covers the first)
        out_t = const_pool.tile([B_PER, D], mybir.dt.float32)
        nc.vector.tensor_copy(out=out_t[:, 0:HALF], in_=ps0[:])
        nc.vector.tensor_copy(out=out_t[:, HALF:D], in_=ps1[:])
        nc.sync.dma_start(out=y[:], in_=out_t[:])

    _split_multiwait_drains(nc)
    return nc


def _split_multiwait_drains(nc: bass.Bass) -> None:
    """walrus lowers at most one sync wait per instruction; Tile's kernel-tail
    drain waits on the whole global clock.  Split it into a chain of
    single-wait drains (a drain with nothing new pending is a no-op, and the
    SP sequencer executes the waits in order, which is equivalent)."""
    for blk in nc.m.functions[0].blocks:
        insts = blk.instructions
        k = 0
        while k < len(insts):
            i = insts[k]
            si = i.sync_info
            if si is not None and len(si.on_wait) > 1:
                assert type(i).__name__ == "InstDrain", (i.name, type(i).__name__)
                waits = list(si.on_wait)
                for j, w in enumerate(waits[:-1]):
                    nd = mybir.InstDrain(
                        name=f"{i.name}-wsplit{j}", engine=i.engine, ins=[], outs=[]
                    )
                    nd.sync_info = mybir.SyncInfo(on_wait=[w], on_update=[])
                    nc.register_instruction(nd, overwrite=True)
                    insts.insert(k + j, nd)
                i.sync_info = mybir.SyncInfo(
                    on_wait=[waits[-1]], on_update=list(si.on_update)
                )
                k += len(waits) - 1
            k += 1


def _shards(x0: np.ndarray) -> list[dict[str, np.ndarray]]:
    return [
        {"x": np.ascontiguousarray(x0[i * B_PER : (i + 1) * B_PER])}
        for i in range(N_CORES)
    ]


def kernel(**inputs: np.ndarray) -> np.ndarray:
    x0 = np.asarray(inputs["x0"], dtype=np.float32)
    if "nc" not in _CACHE:
        _CACHE["nc"] = _build()
    res = run_bass_kernel_spmd(_CACHE["nc"], _shards(x0), core_ids=list(range(N_CORES)))
    return np.concatenate([r["y"] for r in res.results], axis=0)
